# revision 8
# baseline (speedup 1.0000x reference)
"""Trainium2 Bass kernel for nn_AudioEncoder (mel-spectrogram -> conv stack -> VQ).

Self-contained: hardcodes all shapes. Shards the batch (16) across 8 cores
(2 rows per core); conv weights, DFT matrices and the codebook are replicated.
"""
import numpy as np
from contextlib import ExitStack

import concourse.bass as bass
import concourse.bacc as bacc
import concourse.tile as tile
import concourse.mybir as mybir
from concourse import bass_utils

F32R = mybir.dt.float32r
F32 = mybir.dt.float32
I32 = mybir.dt.int32
U32 = mybir.dt.uint32

B, S = 16, 524288
N_FFT, HOP, N_MELS = 2048, 512, 128
NF = 1025            # STFT frames per batch row
NFREQ = 1024         # usable freq bins (mel_fb row 1024 is ~0 -> skipped)
D, V = 512, 4096
L = 513              # conv output length
NCORES = 8
RPC = B // NCORES    # rows per core
XPLEN = S + N_FFT    # padded waveform length = 526336 = 1028*512
QM = XPLEN // HOP    # 1028
QTW = QM + 2         # QT width: col 1028 zeroed (read by the garbage frame)

# frame tiles: fp32r matmul needs EVEN moving dim, so compute 1026 frames
# (the 1026th is garbage; its column is re-zeroed before the next stage reads it)
T_TILES = [(0, 342), (342, 342), (684, 342)]
# conv2/conv3 output tiles: 514 outputs, the 514th is garbage
T2_TILES = [(0, 258), (258, 256)]
# VQ token tiles over 2*513 = 1026 columns of concatenated h3
TOK_TILES = [(i * 128, 128) for i in range(8)] + [(1024, 2)]

_CACHE = {}


def _build_program():
    nc = bacc.Bacc("TRN2", target_bir_lowering=False, debug=False,
                   enable_asserts=True, num_devices=NCORES)

    # ---- DRAM tensors -------------------------------------------------
    d_xp = nc.dram_tensor("xp", (RPC, XPLEN), F32R, kind="ExternalInput").ap()
    d_dftC = nc.dram_tensor("dftC", (N_FFT, NFREQ), F32R, kind="ExternalInput").ap()
    d_dftS = nc.dram_tensor("dftS", (N_FFT, NFREQ), F32R, kind="ExternalInput").ap()
    d_fbT = nc.dram_tensor("fbT", (NFREQ, N_MELS), F32R, kind="ExternalInput").ap()
    d_w1T = nc.dram_tensor("w1T", (3, 128, 256), F32R, kind="ExternalInput").ap()
    d_w2T = nc.dram_tensor("w2T", (3, 256, 512), F32R, kind="ExternalInput").ap()
    d_w3T = nc.dram_tensor("w3T", (3, 512, 512), F32R, kind="ExternalInput").ap()
    d_b1 = nc.dram_tensor("b1", (2, 128), F32, kind="ExternalInput").ap()
    d_b2 = nc.dram_tensor("b2", (4, 128), F32, kind="ExternalInput").ap()
    d_b3 = nc.dram_tensor("b3", (4, 128), F32, kind="ExternalInput").ap()
    d_cbT2 = nc.dram_tensor("cbT2", (D, V), F32R, kind="ExternalInput").ap()
    d_negc2 = nc.dram_tensor("negc2", (1, V), F32R, kind="ExternalInput").ap()
    d_ones = nc.dram_tensor("ones1", (1, 128), F32R, kind="ExternalInput").ap()
    d_ident = nc.dram_tensor("ident", (128, 128), F32R, kind="ExternalInput").ap()
    d_zeros = nc.dram_tensor("zeros4", (128, 4), F32R, kind="ExternalInput").ap()
    d_cb = nc.dram_tensor("codebook", (V, D), F32, kind="ExternalInput").ap()

    d_emb = nc.dram_tensor("emb", (RPC, L, D), F32, kind="ExternalOutput").ap()
    d_tok = nc.dram_tensor("tok", (RPC, L), I32, kind="ExternalOutput").ap()

    emb_flat = d_emb.rearrange("b l d -> (b l) d")
    tok_flat = d_tok.rearrange("b l -> (b l)")
    xp_q = d_xp.rearrange("b (m j) -> b m j", j=HOP)        # (2, 1028, 512)
    dftC_v = d_dftC.rearrange("(k p) f -> p k f", p=128)    # (128, 16, 1024)
    dftS_v = d_dftS.rearrange("(k p) f -> p k f", p=128)

    with tile.TileContext(nc) as tc, ExitStack() as octx:
        Gelu = mybir.ActivationFunctionType.Gelu
        Ln = mybir.ActivationFunctionType.Ln

        const = octx.enter_context(tc.tile_pool(name="const", bufs=1))
        ident = const.tile([128, 128], F32R, tag="ident", name="ident")
        nc.sync.dma_start(ident[:], d_ident)
        ones1 = const.tile([1, 128], F32R, tag="ones1", name="ones1")
        nc.sync.dma_start(ones1[:], d_ones)
        zeros4 = const.tile([128, 4], F32R, tag="zeros4", name="zeros4")
        nc.sync.dma_start(zeros4[:], d_zeros)

        # X1 (log-mel, conv1 input) outlives the spectrogram stage
        x1_pool = octx.enter_context(tc.tile_pool(name="x1", bufs=1))
        X1 = [x1_pool.tile([128, NF + 3], F32R, tag=f"x1_{r}", name=f"x1_{r}") for r in range(RPC)]

        # ============== Stage A/B: frames -> DFT -> power -> mel -> log =====
        with ExitStack() as sctx:
            qt_pool = sctx.enter_context(tc.tile_pool(name="qt", bufs=1))
            qin_pool = sctx.enter_context(tc.tile_pool(name="qin", bufs=3))
            tp_ps = sctx.enter_context(tc.tile_pool(name="tp_ps", bufs=2, space="PSUM"))
            spec_pool = sctx.enter_context(tc.tile_pool(name="spec", bufs=1))
            cs_pool = sctx.enter_context(tc.tile_pool(name="cs", bufs=2))
            dft_ps = sctx.enter_context(tc.tile_pool(name="dft_ps", bufs=2, space="PSUM"))
            sq_pool = sctx.enter_context(tc.tile_pool(name="sq", bufs=3))
            fb_pool = sctx.enter_context(tc.tile_pool(name="fb", bufs=1))
            mel_ps = sctx.enter_context(tc.tile_pool(name="mel_ps", bufs=2, space="PSUM"))

            # Q^T: per row, 4 tiles of (128 j, 1028 m); QT[jb][j, m] = xp[512*m + 128*jb + j]
            QT = [[qt_pool.tile([128, QTW], F32R, tag=f"qt_{r}_{jb}", name=f"qt_{r}_{jb}") for jb in range(4)]
                  for r in range(RPC)]
            for r in range(RPC):
                for jb in range(4):
                    nc.sync.dma_start(QT[r][jb][:, QM:QTW], d_zeros[:, 0:2])
                for mb in range(8):
                    qin = qin_pool.tile([128, HOP], F32R, tag="qin", name="qin")
                    nc.sync.dma_start(qin[:], xp_q[r, mb * 128:(mb + 1) * 128, :])
                    for jb in range(4):
                        pt = tp_ps.tile([128, 128], F32R, tag="pt", name="pt")
                        nc.tensor.transpose(pt[:], qin[:, jb * 128:(jb + 1) * 128], ident[:])
                        nc.scalar.copy(QT[r][jb][:, mb * 128:(mb + 1) * 128], pt[:])
                # tail: rows 1024..1027 of the (1028, 512) view
                qin4 = qin_pool.tile([4, HOP], F32R, tag="qin4", name="qin4")
                nc.sync.dma_start(qin4[:], xp_q[r, 1024:1028, :])
                for jb in range(4):
                    pt4 = tp_ps.tile([128, 128], F32R, tag="pt", name="pt4")
                    nc.tensor.transpose(pt4[:, :4], qin4[:, jb * 128:(jb + 1) * 128],
                                        ident[:4, :4])
                    nc.scalar.copy(QT[r][jb][:, 1024:1028], pt4[:, :4])

            # spectrogram power, (freq chunk, frames) layout
            SPEC = [[spec_pool.tile([128, NF + 1], F32R, tag=f"spec_{m}_{r}", name=f"spec_{m}_{r}")
                     for r in range(RPC)] for m in range(8)]
            for m in range(8):
                cm = cs_pool.tile([128, 16, 128], F32R, tag="cm", name="cm")
                sm = cs_pool.tile([128, 16, 128], F32R, tag="sm", name="sm")
                nc.sync.dma_start(cm[:], dftC_v[:, :, m * 128:(m + 1) * 128])
                nc.sync.dma_start(sm[:], dftS_v[:, :, m * 128:(m + 1) * 128])
                for r in range(RPC):
                    for (t0, nt) in T_TILES:
                        psR = dft_ps.tile([128, 342], F32, tag="psR", name="psR")
                        psI = dft_ps.tile([128, 342], F32, tag="psI", name="psI")
                        for k in range(16):
                            a, jb = k // 4, k % 4
                            rhs = QT[r][jb][:, t0 + a: t0 + a + nt]
                            nc.tensor.matmul(psR[:, :nt], cm[:, k, :], rhs,
                                             start=(k == 0), stop=(k == 15))
                        for k in range(16):
                            a, jb = k // 4, k % 4
                            rhs = QT[r][jb][:, t0 + a: t0 + a + nt]
                            nc.tensor.matmul(psI[:, :nt], sm[:, k, :], rhs,
                                             start=(k == 0), stop=(k == 15))
                        q1 = sq_pool.tile([128, 342], F32, tag="q1", name="q1")
                        q2 = sq_pool.tile([128, 342], F32, tag="q2", name="q2")
                        nc.scalar.square(q1[:, :nt], psR[:, :nt])
                        nc.scalar.square(q2[:, :nt], psI[:, :nt])
                        nc.vector.tensor_add(SPEC[m][r][:, t0:t0 + nt],
                                             q1[:, :nt], q2[:, :nt])

            # mel projection + clip + log
            FB = [fb_pool.tile([128, 128], F32R, tag=f"fb_{kf}", name=f"fb_{kf}") for kf in range(8)]
            for kf in range(8):
                nc.sync.dma_start(FB[kf][:], d_fbT[kf * 128:(kf + 1) * 128, :])
            for r in range(RPC):
                nc.sync.dma_start(X1[r][:, 0:1], d_zeros[:, 0:1])
                nc.sync.dma_start(X1[r][:, NF + 2:NF + 3], d_zeros[:, 0:1])
                for (t0, nt) in T_TILES:
                    pm = mel_ps.tile([128, 342], F32, tag="pm", name="pm")
                    for kf in range(8):
                        nc.tensor.matmul(pm[:, :nt], FB[kf][:],
                                         SPEC[kf][r][:, t0:t0 + nt],
                                         start=(kf == 0), stop=(kf == 7))
                    cl = sq_pool.tile([128, 342], F32, tag="cl", name="cl")
                    nc.vector.tensor_scalar(cl[:, :nt], pm[:, :nt], 1e-5, None,
                                            op0=mybir.AluOpType.max)
                    nc.scalar.activation(X1[r][:, 1 + t0:1 + t0 + nt], cl[:, :nt], Ln)
                nc.sync.dma_start(X1[r][:, NF + 1:NF + 2], d_zeros[:, 0:1])

        # ============== Stage C: conv stack =================================
        h3_pool = octx.enter_context(tc.tile_pool(name="h3", bufs=1))
        H3 = [h3_pool.tile([128, RPC * L + 1], F32R, tag=f"h3_{co}", name=f"h3_{co}") for co in range(4)]

        with ExitStack() as cctx:
            w_pool = cctx.enter_context(tc.tile_pool(name="wts", bufs=1))
            h_pool = cctx.enter_context(tc.tile_pool(name="hbuf", bufs=1))
            cv_ps = cctx.enter_context(tc.tile_pool(name="cv_ps", bufs=2, space="PSUM"))

            # conv1: (128 -> 256), stride 1
            W1 = [w_pool.tile([128, 256], F32R, tag=f"w1_{k}", name=f"w1_{k}") for k in range(3)]
            for k in range(3):
                nc.sync.dma_start(W1[k][:], d_w1T[k])
            B1 = [w_pool.tile([128, 1], F32, tag=f"b1_{co}", name=f"b1_{co}") for co in range(2)]
            for co in range(2):
                nc.sync.dma_start(B1[co][:], d_b1[co, :])
            H1 = [[h_pool.tile([128, NF + 4], F32R, tag=f"h1_{r}_{co}", name=f"h1_{r}_{co}")
                   for co in range(2)] for r in range(RPC)]
            for r in range(RPC):
                for co in range(2):
                    nc.sync.dma_start(H1[r][co][:, 0:1], d_zeros[:, 0:1])
                    nc.sync.dma_start(H1[r][co][:, NF + 2:NF + 4], d_zeros[:, 0:2])
                    for (t0, nt) in T_TILES:
                        pc = cv_ps.tile([128, 342], F32, tag="pc1", name="pc1")
                        for k in range(3):
                            nc.tensor.matmul(pc[:, :nt],
                                             W1[k][:, co * 128:(co + 1) * 128],
                                             X1[r][:, t0 + k: t0 + k + nt],
                                             start=(k == 0), stop=(k == 2))
                        nc.scalar.activation(H1[r][co][:, 1 + t0:1 + t0 + nt],
                                             pc[:, :nt], Gelu, bias=B1[co][:])
                    nc.sync.dma_start(H1[r][co][:, NF + 1:NF + 2], d_zeros[:, 0:1])

            # conv2: (256 -> 512), stride 2
            W2 = [[w_pool.tile([128, 512], F32R, tag=f"w2_{k}_{ci}", name=f"w2_{k}_{ci}") for ci in range(2)]
                  for k in range(3)]
            for k in range(3):
                for ci in range(2):
                    nc.sync.dma_start(W2[k][ci][:], d_w2T[k, ci * 128:(ci + 1) * 128, :])
            B2 = [w_pool.tile([128, 1], F32, tag=f"b2_{co}", name=f"b2_{co}") for co in range(4)]
            for co in range(4):
                nc.sync.dma_start(B2[co][:], d_b2[co, :])
            H2 = [[h_pool.tile([128, L + 3], F32R, tag=f"h2_{r}_{co}", name=f"h2_{r}_{co}")
                   for co in range(4)] for r in range(RPC)]
            for r in range(RPC):
                for co in range(4):
                    nc.sync.dma_start(H2[r][co][:, 0:1], d_zeros[:, 0:1])
                    nc.sync.dma_start(H2[r][co][:, L + 2:L + 3], d_zeros[:, 0:1])
                    for (t0, nt) in T2_TILES:
                        pc = cv_ps.tile([128, 258], F32, tag="pc2", name="pc2")
                        n_mm = 6
                        i_mm = 0
                        for ci in range(2):
                            for k in range(3):
                                c0 = 2 * t0 + k
                                rhs = H1[r][ci][:, c0: c0 + 2 * nt - 1: 2]
                                nc.tensor.matmul(pc[:, :nt],
                                                 W2[k][ci][:, co * 128:(co + 1) * 128],
                                                 rhs, start=(i_mm == 0),
                                                 stop=(i_mm == n_mm - 1))
                                i_mm += 1
                        nc.scalar.activation(H2[r][co][:, 1 + t0:1 + t0 + nt],
                                             pc[:, :nt], Gelu, bias=B2[co][:])
                    nc.sync.dma_start(H2[r][co][:, L + 1:L + 2], d_zeros[:, 0:1])

            # conv3: (512 -> 512), stride 1; output rows concatenated per co chunk
            W3 = [[w_pool.tile([128, 512], F32R, tag=f"w3_{k}_{ci}", name=f"w3_{k}_{ci}") for ci in range(4)]
                  for k in range(3)]
            for k in range(3):
                for ci in range(4):
                    nc.sync.dma_start(W3[k][ci][:], d_w3T[k, ci * 128:(ci + 1) * 128, :])
            B3 = [w_pool.tile([128, 1], F32, tag=f"b3_{co}", name=f"b3_{co}") for co in range(4)]
            for co in range(4):
                nc.sync.dma_start(B3[co][:], d_b3[co, :])
            for r in range(RPC):
                for co in range(4):
                    for (t0, nt) in T2_TILES:
                        pc = cv_ps.tile([128, 258], F32, tag="pc3", name="pc3")
                        n_mm = 12
                        i_mm = 0
                        for ci in range(4):
                            for k in range(3):
                                rhs = H2[r][ci][:, t0 + k: t0 + k + nt]
                                nc.tensor.matmul(pc[:, :nt],
                                                 W3[k][ci][:, co * 128:(co + 1) * 128],
                                                 rhs, start=(i_mm == 0),
                                                 stop=(i_mm == n_mm - 1))
                                i_mm += 1
                        nc.scalar.activation(H3[co][:, r * L + t0: r * L + t0 + nt],
                                             pc[:, :nt], Gelu, bias=B3[co][:])

        # ============== Stage D: VQ codebook lookup =========================
        with ExitStack() as vctx:
            cb_pool = vctx.enter_context(tc.tile_pool(name="cb", bufs=1))
            vq_ps = vctx.enter_context(tc.tile_pool(name="vq_ps", bufs=4, space="PSUM"))
            dist_pool = vctx.enter_context(tc.tile_pool(name="dist", bufs=2))
            out_pool = vctx.enter_context(tc.tile_pool(name="out", bufs=3))

            CB = [cb_pool.tile([128, V], F32R, tag=f"cb_{ci}", name=f"cb_{ci}") for ci in range(4)]
            for ci in range(4):
                nc.sync.dma_start(CB[ci][:], d_cbT2[ci * 128:(ci + 1) * 128, :])
            NC2 = cb_pool.tile([1, V], F32R, tag="negc2", name="negc2")
            nc.sync.dma_start(NC2[:], d_negc2)

            for (tt0, mt) in TOK_TILES:
                dist = dist_pool.tile([mt, V], F32, tag=f"dist{mt}", name=f"dist{mt}")
                for cc in range(8):
                    pv = vq_ps.tile([mt, 512], F32, tag=f"pv{mt}", name=f"pv{mt}")
                    for ci in range(4):
                        nc.tensor.matmul(pv[:], H3[ci][:, tt0:tt0 + mt],
                                         CB[ci][:, cc * 512:(cc + 1) * 512],
                                         start=(ci == 0), stop=False)
                    nc.tensor.matmul(pv[:], ones1[:, :mt],
                                     NC2[:, cc * 512:(cc + 1) * 512],
                                     start=False, stop=True)
                    nc.scalar.copy(dist[:, cc * 512:(cc + 1) * 512], pv[:])
                mv = out_pool.tile([mt, 8], F32, tag=f"mv{mt}", name=f"mv{mt}")
                mi = out_pool.tile([mt, 8], U32, tag=f"mi{mt}", name=f"mi{mt}")
                nc.vector.max(mv[:], dist[:])
                nc.vector.max_index(mi[:], mv[:], dist[:])
                ti = out_pool.tile([mt, 1], I32, tag=f"ti{mt}", name=f"ti{mt}")
                nc.vector.tensor_copy(ti[:], mi[:, 0:1])
                nc.sync.dma_start(tok_flat[tt0:tt0 + mt], ti[:])
                gt = out_pool.tile([mt, D], F32, tag=f"gt{mt}", name=f"gt{mt}")
                nc.gpsimd.indirect_dma_start(
                    out=gt[:], out_offset=None, in_=d_cb,
                    in_offset=bass.IndirectOffsetOnAxis(ap=ti[:, 0:1], axis=0))
                nc.sync.dma_start(emb_flat[tt0:tt0 + mt, :], gt[:])

    nc.compile()
    return nc


def _host_consts(mel_fb, w1, b1, w2, b2, w3, b3, codebook):
    n32 = np.arange(N_FFT, dtype=np.float32)
    window = (np.float32(0.5) - np.float32(0.5) *
              np.cos(np.float32(2.0 * np.pi) * n32 / np.float32(N_FFT)))
    n = np.arange(N_FFT, dtype=np.float64)
    f = np.arange(NFREQ, dtype=np.float64)
    ang = (2.0 * np.pi / N_FFT) * np.outer(n, f)
    dftC = (np.cos(ang) * window.astype(np.float64)[:, None]).astype(np.float32)
    dftS = (-np.sin(ang) * window.astype(np.float64)[:, None]).astype(np.float32)

    consts = {
        "dftC": np.ascontiguousarray(dftC),
        "dftS": np.ascontiguousarray(dftS),
        "fbT": np.ascontiguousarray(mel_fb[:NFREQ, :].astype(np.float32)),
        "w1T": np.ascontiguousarray(np.transpose(w1, (2, 1, 0)).astype(np.float32)),
        "w2T": np.ascontiguousarray(np.transpose(w2, (2, 1, 0)).astype(np.float32)),
        "w3T": np.ascontiguousarray(np.transpose(w3, (2, 1, 0)).astype(np.float32)),
        "b1": np.ascontiguousarray(b1.astype(np.float32).reshape(2, 128)),
        "b2": np.ascontiguousarray(b2.astype(np.float32).reshape(4, 128)),
        "b3": np.ascontiguousarray(b3.astype(np.float32).reshape(4, 128)),
        "cbT2": np.ascontiguousarray((2.0 * codebook.astype(np.float32)).T),
        "negc2": np.ascontiguousarray(
            (-np.sum(codebook.astype(np.float64) ** 2, axis=1))
            .astype(np.float32).reshape(1, V)),
        "ones1": np.ones((1, 128), np.float32),
        "ident": np.eye(128, dtype=np.float32),
        "zeros4": np.zeros((128, 4), np.float32),
        "codebook": np.ascontiguousarray(codebook.astype(np.float32)),
    }
    return consts


def kernel(waveform, mel_fb, w1, b1, w2, b2, w3, b3, codebook, **kwargs):
    waveform = np.asarray(waveform, dtype=np.float32)
    consts = _host_consts(np.asarray(mel_fb), np.asarray(w1), np.asarray(b1),
                          np.asarray(w2), np.asarray(b2), np.asarray(w3),
                          np.asarray(b3), np.asarray(codebook))

    if "nc" not in _CACHE:
        _CACHE["nc"] = _build_program()
    nc = _CACHE["nc"]

    pad = N_FFT // 2
    xp = np.pad(waveform, ((0, 0), (pad, pad)), mode="reflect")

    in_maps = []
    for c in range(NCORES):
        m = dict(consts)
        m["xp"] = np.ascontiguousarray(xp[c * RPC:(c + 1) * RPC, :])
        in_maps.append(m)

    _CACHE["last_in_maps"] = in_maps
    res = bass_utils.run_bass_kernel_spmd(nc, in_maps, core_ids=list(range(NCORES)))

    emb = np.concatenate([res.results[c]["emb"] for c in range(NCORES)], axis=0)
    tok = np.concatenate([res.results[c]["tok"] for c in range(NCORES)], axis=0)
    return emb.astype(np.float32), tok.astype(np.int32)
